# revision 1
# baseline (speedup 1.0000x reference)
"""Trainium2 Bass kernel for sliding-window causal self-attention (GQA + RoPE +
RMS-norm QK + value-embedding gating).

Sharding: 8 cores = 2 (batch) x 4 (KV groups).  Each core handles one batch
element and one KV head (= 4 query heads), computes a partial output through
the row-slice of Wproj for its heads; the host sums the 4 partials per batch.

v2: 4-head-batched attention (FD=512 ops), RMS recip via Ln/Exp (single
activation table set), gate sigmoid via Exp+reciprocal, RoPE in bf16 SBUF,
Q normalization folded before RoPE (linearity), bf16 output.
"""

import sys
import os

for _p in ("/root/.axon_site", "/root/.axon_site/_ro/trn_rl_repo",
           "/root/.axon_site/_ro/pypackages", "/opt/trn_rl_repo"):
    if os.path.isdir(_p) and _p not in sys.path:
        sys.path.append(_p)

import numpy as np
import ml_dtypes
from contextlib import ExitStack

import concourse.bass as bass
import concourse.tile as tile
from concourse import bacc, mybir
from concourse.bass_utils import run_bass_kernel_spmd

BF16 = ml_dtypes.bfloat16
N_HEAD, N_KV, HEAD_DIM, WINDOW, N_EMBD = 16, 4, 64, 512, 1024
B, T = 2, 2048
NCORES = 8
TCH = 512               # token chunk for the projection phase
NCH = T // TCH          # 4
NTT = T // 128          # 16 t-tiles

F32 = mybir.dt.float32
BF = mybir.dt.bfloat16
AF = mybir.ActivationFunctionType
OP = mybir.AluOpType

_cache = {}


def _build():
    nc = bacc.Bacc("TRN2", target_bir_lowering=False, debug=False,
                   num_devices=NCORES)

    xt_d = nc.dram_tensor("xt", [8, 128, T], BF, kind="ExternalInput")
    wq_d = nc.dram_tensor("wq", [128, 8 * 256], BF, kind="ExternalInput")
    wkv_d = nc.dram_tensor("wkv", [128, 8 * 128], BF, kind="ExternalInput")
    wg_d = nc.dram_tensor("wg", [32, 1], BF, kind="ExternalInput")
    wp_d = nc.dram_tensor("wp", [128, 2 * 1024], BF, kind="ExternalInput")
    cs1_d = nc.dram_tensor("cs1", [128, T], BF, kind="ExternalInput")
    cs2_d = nc.dram_tensor("cs2", [128, T], BF, kind="ExternalInput")
    ve_d = nc.dram_tensor("ve2", [128, 16 * 64], BF, kind="ExternalInput")
    msk_d = nc.dram_tensor("masks", [128, 1024], BF, kind="ExternalInput")
    id_d = nc.dram_tensor("ident", [64, 64], BF, kind="ExternalInput")
    selq_d = nc.dram_tensor("selq2", [128, 2], BF, kind="ExternalInput")
    sel128_d = nc.dram_tensor("sel128", [2, 128], BF, kind="ExternalInput")
    on64_d = nc.dram_tensor("ones64", [64, 1], BF, kind="ExternalInput")
    on1x_d = nc.dram_tensor("ones1x64", [1, 64], BF, kind="ExternalInput")
    id1_d = nc.dram_tensor("id1", [1, 1], BF, kind="ExternalInput")
    mab_d = nc.dram_tensor("mab", [128, 256], BF, kind="ExternalInput")
    out_d = nc.dram_tensor("out", [T, N_EMBD], BF, kind="ExternalOutput")

    with tile.TileContext(nc) as tc, ExitStack() as ctx:
        pers = ctx.enter_context(tc.tile_pool(name="pers", bufs=1))
        wk = ctx.enter_context(tc.tile_pool(name="wk", bufs=10))
        ptp = ctx.enter_context(tc.tile_pool(name="ptp", bufs=8))
        sm = ctx.enter_context(tc.tile_pool(name="sm", bufs=6))
        ow = ctx.enter_context(tc.tile_pool(name="ow", bufs=4))
        evp = ctx.enter_context(tc.tile_pool(name="evp", bufs=15))
        pbA = ctx.enter_context(tc.tile_pool(name="pbA", bufs=2, space="PSUM"))
        pbSt = ctx.enter_context(tc.tile_pool(name="pbSt", bufs=3, space="PSUM"))
        pbY = ctx.enter_context(tc.tile_pool(name="pbY", bufs=1, space="PSUM"))
        pbS = ctx.enter_context(tc.tile_pool(name="pbS", bufs=2, space="PSUM"))

        # ---- persistent SBUF loads ----
        xt_sb = []
        qs = [nc.sync, nc.gpsimd, nc.scalar]
        for k in range(8):
            t_ = pers.tile([128, T], BF, tag=f"xt{k}", name=f"xt{k}")
            qs[k % 3].dma_start(t_[:], xt_d[k])
            xt_sb.append(t_)
        cs1_sb = pers.tile([128, T], BF, tag="cs1")
        nc.gpsimd.dma_start(cs1_sb[:], cs1_d[:])
        cs2_sb = pers.tile([128, T], BF, tag="cs2")
        nc.scalar.dma_start(cs2_sb[:], cs2_d[:])
        wq_sb = pers.tile([128, 8 * 256], BF, tag="wq")
        nc.gpsimd.dma_start(wq_sb[:], wq_d[:])
        wkv_sb = pers.tile([128, 8 * 128], BF, tag="wkv")
        nc.gpsimd.dma_start(wkv_sb[:], wkv_d[:])
        wg_sb = pers.tile([32, 1], BF, tag="wg")
        nc.sync.dma_start(wg_sb[:], wg_d[:])
        ve_sb = pers.tile([128, 16 * 64], BF, tag="ve")
        nc.sync.dma_start(ve_sb[:], ve_d[:])
        mskc_sb = pers.tile([128, 512], BF, tag="mskc")
        nc.sync.dma_start(mskc_sb[:], msk_d[:, 0:512])
        mskw_sb = pers.tile([128, 512], BF, tag="mskw")
        nc.sync.dma_start(mskw_sb[:], msk_d[:, 512:1024])
        id_sb = pers.tile([64, 64], BF, tag="ident")
        nc.sync.dma_start(id_sb[:], id_d[:])
        selq_sb = pers.tile([128, 2], BF, tag="selq2")
        nc.sync.dma_start(selq_sb[:], selq_d[:])
        sel128_sb = pers.tile([2, 128], BF, tag="sel128")
        nc.sync.dma_start(sel128_sb[:], sel128_d[:])
        on64_sb = pers.tile([64, 1], BF, tag="on64")
        nc.sync.dma_start(on64_sb[:], on64_d[:])
        on1x_sb = pers.tile([1, 64], BF, tag="on1x")
        nc.sync.dma_start(on1x_sb[:], on1x_d[:])
        id1_sb = pers.tile([1, 1], BF, tag="id1")
        nc.sync.dma_start(id1_sb[:], id1_d[:])
        ma_sb = pers.tile([128, 128], BF, tag="ma")
        nc.sync.dma_start(ma_sb[:], mab_d[:, 0:128])
        mb_sb = pers.tile([128, 128], BF, tag="mb")
        nc.sync.dma_start(mb_sb[:], mab_d[:, 128:256])
        wp_sb = pers.tile([128, 2 * 1024], BF, tag="wp")
        nc.gpsimd.dma_start(wp_sb[:], wp_d[:])

        # ---- persistent intermediates ----
        # Q^T, 4 heads side-by-side per q-tile: [64, qt(16) x h(4) x 128]
        q4t = pers.tile([64, NTT * 512], BF, tag="q4t")
        kt_sb = pers.tile([64, T], BF, tag="kt")      # K^T (un-normalized)
        vn_sb = pers.tile([128, NTT * 65], BF, tag="vn")  # V natural + ones col
        # y^T: [128 (2 heads stacked), p(2) x T]
        yt_sb = pers.tile([128, 2 * T], BF, tag="yt")
        rk_sb = pers.tile([128, NTT], F32, tag="rk")  # K rms recip, natural
        g_sb = pers.tile([128, NTT], F32, tag="g")    # sigmoid gates, natural

        nc.vector.memset(vn_sb[:], 1.0)      # ones columns (col 64 of each group)
        biasq_sb = pers.tile([2, 1], F32, tag="biasq")
        nc.vector.memset(biasq_sb[:], 64e-6)
        biask_sb = pers.tile([1, 1], F32, tag="biask")
        nc.vector.memset(biask_sb[:], 1e-6)

        # ---- gates for all t-tiles (sigmoid via exp + reciprocal) ----
        gps = pbS.tile([128, NTT], F32, tag="s", name="gps")
        for tt in range(NTT):
            nc.tensor.matmul(gps[:, tt:tt + 1],
                             xt_sb[0][0:32, tt * 128:(tt + 1) * 128],
                             wg_sb[:], start=True, stop=True)
        eg = sm.tile([128, NTT], F32, tag="u", name="eg")
        nc.scalar.activation(eg[:], gps[:], AF.Exp, scale=-1.0)
        eg1 = sm.tile([128, NTT], F32, tag="u", name="eg1")
        nc.vector.tensor_scalar_add(eg1[:], eg[:], 1.0)
        nc.vector.reciprocal(g_sb[:], eg1[:])

        def qkv_matmul(psum, w_sb, col0, ncol, c0):
            for k in range(8):
                nc.tensor.matmul(
                    psum, w_sb[:, k * ncol + col0: k * ncol + col0 + 128],
                    xt_sb[k][:, c0:c0 + TCH],
                    start=(k == 0), stop=(k == 7))

        def chunk_mms(ch):
            """QKV matmuls (dense PE block) + immediate PSUM evacuation."""
            c0 = ch * TCH
            tl = {"c0": c0}
            psqs = []
            for p in range(2):
                psq = pbA.tile([128, TCH], F32, tag="A", name="psq")
                qkv_matmul(psq, wq_sb, p * 128, 256, c0)
                psqs.append(psq)
            pskv = pbY.tile([128, TCH], F32, tag="y", name="pskv")
            qkv_matmul(pskv, wkv_sb, 0, 128, c0)
            for p in range(2):
                sq = evp.tile([128, TCH], BF, tag="e", name="sq")
                nc.scalar.square(sq[:], psqs[p][:])
                pb = evp.tile([128, TCH], BF, tag="e", name="pb")
                nc.scalar.copy(pb[:], psqs[p][:])
                tl["sq%d" % p] = sq
                tl["pb%d" % p] = pb
            sqk = evp.tile([64, TCH], BF, tag="e", name="sqk")
            nc.scalar.square(sqk[:], pskv[0:64])
            pbk = evp.tile([64, TCH], BF, tag="e", name="pbk")
            nc.scalar.copy(pbk[:], pskv[0:64])
            vt = evp.tile([64, TCH], BF, tag="e", name="vt")
            nc.scalar.copy(vt[:], pskv[64:128])
            tl.update(sqk=sqk, pbk=pbk, vt=vt)
            return tl

        def chunk_epilogue(ch, tl):
            c0 = tl["c0"]
            csl = slice(c0, c0 + TCH)
            q4v = q4t[:, ch * 2048:(ch + 1) * 2048].rearrange(
                "p (j h c) -> p j h c", j=4, h=4, c=128)
            for p in range(2):
                sq, pb = tl["sq%d" % p], tl["pb%d" % p]
                ss = pbS.tile([2, TCH], F32, tag="s", name="ss")
                nc.tensor.matmul(ss[:], selq_sb[:], sq[:], start=True,
                                 stop=True)
                srt = sm.tile([2, TCH], F32, tag="u", name="srtq")
                nc.scalar.activation(srt[:], ss[:], AF.Sqrt, bias=biasq_sb[:],
                                     scale=1.0)
                rcpf = sm.tile([2, TCH], F32, tag="rf", name="rcpf")
                nc.vector.reciprocal_approx_fast(rcpf[:], srt[:])
                rcpb = sm.tile([2, TCH], BF, tag="rc", name="rcpb")
                nc.scalar.copy(rcpb[:], rcpf[:])
                bcps = pbSt.tile([128, TCH], F32, tag="st", name="bcps")
                nc.tensor.matmul(bcps[:], sel128_sb[:], rcpb[:], start=True,
                                 stop=True)
                bcs = wk.tile([128, TCH], BF, tag="w", name="bcs")
                nc.scalar.copy(bcs[:], bcps[:])
                pbn = wk.tile([128, TCH], BF, tag="w", name="pbn")
                nc.vector.tensor_mul(pbn[:], pb[:], bcs[:])
                # A = pbn*cs1 -> [x1c, x2s]; P2 = pbn*cs2 -> [x1s, x2c];
                # rope combine on PE: ro = MA^T @ A + MB^T @ P2
                A = wk.tile([128, TCH], BF, tag="w", name="ropeA")
                P2 = wk.tile([128, TCH], BF, tag="w", name="ropeP2")
                nc.vector.tensor_mul(A[:], pbn[:], cs1_sb[:, csl])
                nc.vector.tensor_mul(P2[:], pbn[:], cs2_sb[:, csl])
                ro = pbSt.tile([128, TCH], F32, tag="st", name="ro")
                nc.tensor.matmul(ro[:], ma_sb[:], A[:], start=True, stop=False)
                nc.tensor.matmul(ro[:], mb_sb[:], P2[:], start=False, stop=True)
                for i in range(2):
                    h = 2 * p + i
                    nc.vector.tensor_copy(q4v[:, :, h, :],
                                          ro[64 * i:64 * i + 64])
            # ---------------- K | V^T ----------------
            sqk, pbk, vt = tl["sqk"], tl["pbk"], tl["vt"]
            ssk = pbS.tile([1, TCH], F32, tag="s", name="ssk")
            nc.tensor.matmul(ssk[:], on64_sb[:], sqk[:], start=True, stop=True)
            srtk = sm.tile([1, TCH], F32, tag="u", name="srtk")
            nc.scalar.activation(srtk[:], ssk[:], AF.Sqrt, bias=biask_sb[:],
                                 scale=1.0 / 64)
            rkf = sm.tile([1, TCH], F32, tag="rf", name="rkf")
            nc.vector.reciprocal_approx_fast(rkf[:], srtk[:])
            rkb = sm.tile([1, TCH], BF, tag="rc", name="rkb")
            nc.scalar.copy(rkb[:], rkf[:])
            for j in range(4):
                tt = ch * 4 + j
                rkp = pbS.tile([128, 1], BF, tag="s", name="rkp")
                nc.tensor.transpose(rkp[:], rkb[:, j * 128:(j + 1) * 128],
                                    id1_sb[:])
                nc.scalar.copy(rk_sb[:, tt:tt + 1], rkp[:])
            Ak = wk.tile([64, TCH], BF, tag="w", name="ropeAk")
            Pk = wk.tile([64, TCH], BF, tag="w", name="ropePk")
            nc.vector.tensor_mul(Ak[:], pbk[:], cs1_sb[0:64, csl])
            nc.vector.tensor_mul(Pk[:], pbk[:], cs2_sb[0:64, csl])
            rok = pbSt.tile([64, TCH], F32, tag="st", name="rok")
            nc.tensor.matmul(rok[:], ma_sb[0:64, 0:64], Ak[:], start=True,
                             stop=False)
            nc.tensor.matmul(rok[:], mb_sb[0:64, 0:64], Pk[:], start=False,
                             stop=True)
            nc.vector.tensor_copy(kt_sb[:, csl], rok[:])
            # V natural (+ gate * ve) per t-tile
            for j in range(4):
                tt = ch * 4 + j
                vtp = pbS.tile([128, 64], BF, tag="s", name="vtp")
                nc.tensor.transpose(vtp[:], vt[:, j * 128:(j + 1) * 128],
                                    id_sb[:])
                nc.vector.scalar_tensor_tensor(
                    vn_sb[:, tt * 65: tt * 65 + 64],
                    ve_sb[:, tt * 64:(tt + 1) * 64], g_sb[:, tt:tt + 1],
                    vtp[:], op0=OP.mult, op1=OP.add)

        tl_prev = None
        for ch in range(NCH):
            tl = chunk_mms(ch)
            if tl_prev is not None:
                chunk_epilogue(ch - 1, tl_prev)
            tl_prev = tl
        chunk_epilogue(NCH - 1, tl_prev)

        # ============ attention + projection (all q-tiles) ============
        if True:
            def emit_st(qt):
                lo = max(0, qt - 4)
                q_ap = q4t[:, qt * 512:(qt + 1) * 512]
                sts = {}
                for kt in range(lo, qt + 1):
                    st = pbSt.tile([128, TCH], F32, tag="st", name="st")
                    nc.tensor.matmul(st[:],
                                     kt_sb[:, kt * 128:(kt + 1) * 128], q_ap,
                                     start=True, stop=True)
                    sts[kt] = st
                return sts

            sts_next = emit_st(0)
            for qt in range(NTT):
                lo = max(0, qt - 4)
                sts = sts_next
                if qt + 1 < NTT:
                    sts_next = emit_st(qt + 1)
                yext = pbY.tile([65, TCH], F32, tag="y", name="yext")
                for kt in range(lo, qt + 1):
                    pt = ptp.tile([128, TCH], BF, tag="pt", name="pt")
                    nc.scalar.activation(pt[:], sts[kt][:], AF.Exp,
                                         scale=rk_sb[:, kt:kt + 1])
                    if kt == qt:
                        nc.vector.tensor_mul(pt[:], pt[:], mskc_sb[:])
                    elif kt == qt - 4:
                        nc.vector.tensor_mul(pt[:], pt[:], mskw_sb[:])
                    nc.tensor.matmul(yext[:],
                                     vn_sb[:, kt * 65: kt * 65 + 65], pt[:],
                                     start=(kt == lo), stop=(kt == qt))
                dd = sm.tile([1, TCH], F32, tag="dd", name="dd")
                nc.vector.tensor_copy(dd[:], yext[64:65, :])
                rrf = sm.tile([1, TCH], F32, tag="rf", name="rrf")
                nc.vector.reciprocal_approx_fast(rrf[:], dd[:])
                rrb = sm.tile([1, TCH], BF, tag="rc", name="rrb")
                nc.vector.tensor_copy(rrb[:], rrf[:])
                bcq = pbS.tile([64, TCH], F32, tag="s", name="bcq")
                nc.tensor.matmul(bcq[:], on1x_sb[:], rrb[:], start=True,
                                 stop=True)
                bca = ow.tile([64, TCH], BF, tag="bca", name="bca")
                nc.vector.tensor_copy(bca[:], bcq[:])
                for h in range(4):
                    p, hh = h // 2, (h % 2) * 64
                    nc.vector.tensor_mul(
                        yt_sb[hh:hh + 64, p * T + qt * 128: p * T + (qt + 1) * 128],
                        yext[0:64, h * 128:(h + 1) * 128],
                        bca[:, h * 128:(h + 1) * 128])
                # output projection, pipelined one q-tile behind so the PE
                # never stalls on this q-tile's normalization chain
                for pq in ([qt - 1] if qt >= 1 else []) + ([NTT - 1] if qt == NTT - 1 else []):
                    for cc in range(2):
                        ops = pbA.tile([128, TCH], F32, tag="A", name="ops")
                        for p in range(2):
                            nc.tensor.matmul(
                                ops[:], yt_sb[:, p * T + pq * 128: p * T + (pq + 1) * 128],
                                wp_sb[:, p * 1024 + cc * 512: p * 1024 + cc * 512 + 512],
                                start=(p == 0), stop=(p == 1))
                        o_sb = ow.tile([128, TCH], BF, tag="o", name="osb")
                        if cc == 0:
                            nc.scalar.copy(o_sb[:], ops[:])
                        else:
                            nc.vector.tensor_copy(o_sb[:], ops[:])
                        (nc.sync if cc == 0 else nc.gpsimd).dma_start(
                            out_d[pq * 128:(pq + 1) * 128, cc * 512:(cc + 1) * 512],
                            o_sb[:])

    nc.compile()
    return nc


def _prep_inputs(x, ve, cos, sin, Wq, Wk, Wv, Wproj, Wgate):
    """Build the 8 per-core input maps (host-side sharding + layout prep)."""
    cosT = np.ascontiguousarray(cos.T).astype(np.float32)   # [32, T]
    sinT = np.ascontiguousarray(sin.T).astype(np.float32)
    cs1 = np.concatenate([cosT, sinT, cosT, sinT], 0).astype(BF16)  # [128, T]
    cs2 = np.concatenate([sinT, cosT, sinT, cosT], 0).astype(BF16)
    triu = np.triu(np.ones((128, 128), np.float32))
    tril = np.tril(np.ones((128, 128), np.float32))
    masks = np.concatenate([np.tile(triu, (1, 4)), np.tile(tril, (1, 4))],
                           1).astype(BF16)                  # [128, 1024]
    ident = np.eye(64, dtype=BF16)
    selq2 = np.zeros((128, 2), np.float32)
    selq2[0:64, 0] = 1.0
    selq2[64:128, 1] = 1.0
    selq2 = selq2.astype(BF16)
    sel128 = np.zeros((2, 128), np.float32)
    sel128[0, 0:64] = 1.0
    sel128[1, 64:128] = 1.0
    sel128 = sel128.astype(BF16)
    ones64 = np.ones((64, 1), BF16)
    ones1x64 = np.ones((1, 64), BF16)
    id1 = np.ones((1, 1), BF16)
    # rope combine shuffle matrices: ro = MA^T @ (x*cs1) + MB^T @ (x*cs2)
    MA = np.zeros((128, 128), np.float32)
    MB = np.zeros((128, 128), np.float32)
    for hh in (0, 64):
        for j in range(32):
            m = hh + j
            MA[m, m] = 1.0
            MA[m + 32, m] = -1.0
            m2 = hh + 32 + j
            MB[hh + j, m2] = 1.0
            MB[m2, m2] = 1.0
    mab = np.concatenate([MA, MB], 1).astype(BF16)

    xT = [np.ascontiguousarray(x[b].astype(BF16).T).reshape(8, 128, T)
          for b in range(B)]
    in_maps = []
    for c in range(NCORES):
        b, g = c // 4, c % 4
        wq_g = np.ascontiguousarray(np.transpose(
            Wq[:, g * 256:(g + 1) * 256].reshape(8, 128, 256),
            (1, 0, 2)).reshape(128, 8 * 256)).astype(BF16)
        wkv_g = np.ascontiguousarray(np.transpose(np.concatenate(
            [Wk[:, g * 64:(g + 1) * 64], Wv[:, g * 64:(g + 1) * 64]],
            1).reshape(8, 128, 128), (1, 0, 2)).reshape(128, 8 * 128)
        ).astype(BF16)
        wg_g = np.ascontiguousarray(Wgate[:, g:g + 1]).astype(BF16)
        wp_g = np.ascontiguousarray(np.transpose(
            Wproj[g * 256:(g + 1) * 256, :].reshape(2, 128, 1024),
            (1, 0, 2)).reshape(128, 2 * 1024)).astype(BF16)
        ve_g = np.ascontiguousarray(np.transpose(
            (2.0 * ve[b, :, g * 64:(g + 1) * 64]).reshape(16, 128, 64),
            (1, 0, 2)).reshape(128, 16 * 64)).astype(BF16)
        in_maps.append({
            "xt": xT[b], "wq": wq_g, "wkv": wkv_g, "wg": wg_g, "wp": wp_g,
            "cs1": cs1, "cs2": cs2, "ve2": ve_g, "masks": masks,
            "ident": ident, "selq2": selq2, "sel128": sel128,
            "ones64": ones64, "ones1x64": ones1x64, "id1": id1, "mab": mab,
        })
    return in_maps


def _run(inputs, trace=False, tmpdir=None):
    if "nc" not in _cache:
        _cache["nc"] = _build()
    nc = _cache["nc"]
    in_maps = _prep_inputs(**inputs)
    res = run_bass_kernel_spmd(nc, in_maps, list(range(NCORES)), trace=trace,
                               tmpdir=tmpdir)
    out = np.zeros((B, T, N_EMBD), np.float32)
    for c in range(NCORES):
        out[c // 4] += np.asarray(res.results[c]["out"]).astype(np.float32)
    return out, res


def kernel(**inputs):
    out, _ = _run(inputs)
    return out

